# revision 15
# baseline (speedup 1.0000x reference)
"""Distributed Trainium2 kernel for GQA attention (nn_Attention_76845554860188).

B=1, S=2048, D=1024, NH=16, NKV=4, HD=64, causal, RoPE, 8 NeuronCores.

Sharding: tensor-parallel over heads. Core c owns q-heads {2c, 2c+1} and their
shared GQA kv-head c//2. Each core projects Q/K/V for all 2048 positions,
runs causal flash-style attention for its 2 heads, then ONE AllToAll
redistributes per-head outputs into per-sequence shards; each core applies the
full output projection to its 256-row slice. Host concatenates the 8 slices.

Key performance structure vs the v1 kernel:
  * seq-chunk software pipeline: projection / RoPE / attention are emitted per
    512-column chunk so PE, ACT, DVE and DMA overlap from the start.
  * scores for the two heads run CONCURRENTLY on the PE via row-tiling
    (K=64 each): krot duplicated at partitions 64..127, h1 matmuls auto-derive
    tile_position (64,0) -> disjoint PE quadrants, ~2x score throughput.
  * flash ordering: per q-chunk, score block b+1 streams while exp(b) runs on
    ACT and PV(b) follows - PV accumulates into per-chunk PSUM so no S^2
    probability matrix is materialized (SBUF use drops ~70KB/partition).
  * one 512KB AllToAll replaces four serialized 1MB-out AllGathers (8x less
    wire traffic, no rank-dynamic slicing on the receive side).
  * PE warm-up matmuls during the input DMA fill so HAM reaches full clock
    before the projection matmuls arrive.

On-chip layout is [feature, seq]: scores have k-positions on partitions, so
the softmax denominator comes for free as a ones column appended to V in the
PV matmul. exp() runs on ACT with the 1/sqrt(64) scale folded in; no
max-subtraction is needed (logits are O(6) for unit-scale inputs).
"""

import sys

sys.path.insert(0, "/opt/trn_rl_repo")

import numpy as np
import ml_dtypes

import concourse.bass as bass
import concourse.mybir as mybir
import concourse.tile as tile
from concourse import bacc
from concourse.bass_utils import run_bass_kernel_spmd

BF16 = mybir.dt.bfloat16
F32 = mybir.dt.float32

B, S, D = 1, 2048, 1024
NH, NKV, HD = 16, 4, 64
NC_CORES = 8
HPC = NH // NC_CORES  # q heads per core = 2
SC = S // NC_CORES  # seq slice per core = 256
NDC = D // 128  # d chunks = 8
NSB = S // 128  # 128-wide seq blocks = 16
NCH = S // 512  # 512-wide seq chunks = 4
HALF = HD // 2  # 32

np_bf16 = ml_dtypes.bfloat16


def build_graph(taps=False):
    nc = bacc.Bacc(
        "TRN2", target_bir_lowering=False, debug=False, num_devices=NC_CORES
    )

    # ---- DRAM parameters (per-core shards supplied by host) ----
    xT_e = nc.dram_tensor("xT", [128, NDC, S], BF16, kind="ExternalInput")
    wq_e = nc.dram_tensor("wq", [128, NDC, HPC * HD], BF16, kind="ExternalInput")
    wkv_e = nc.dram_tensor("wkv", [128, NDC, 2 * HD], BF16, kind="ExternalInput")
    wo_e = nc.dram_tensor("wo", [128, NDC, D], BF16, kind="ExternalInput")
    c2_e = nc.dram_tensor("c2", [128, S], BF16, kind="ExternalInput")
    s2_e = nc.dram_tensor("s2", [128, S], BF16, kind="ExternalInput")
    ppm_e = nc.dram_tensor("ppm", [128, 128], BF16, kind="ExternalInput")
    idm_e = nc.dram_tensor("idm", [128, 128], BF16, kind="ExternalInput")
    tri_e = nc.dram_tensor("tri", [128, 128], BF16, kind="ExternalInput")
    out_e = nc.dram_tensor("out", [SC, D], F32, kind="ExternalOutput")

    # internal DRAM bounce buffers for the AllToAll
    send_d = nc.dram_tensor("a2a_send", [NC_CORES, 128, SC], BF16)
    recv_d = nc.dram_tensor("a2a_recv", [NC_CORES, 128, SC], BF16)
    # tiny warmup collective: absorbs the first-collective barrier/setup cost
    wup_s = nc.dram_tensor("wup_s", [NC_CORES, 1, 8], BF16)
    wup_r = nc.dram_tensor("wup_r", [NC_CORES, 1, 8], BF16)

    with tile.TileContext(nc) as tc:
        _body(nc, tc, xT_e, wq_e, wkv_e, wo_e, c2_e, s2_e, ppm_e, idm_e, tri_e,
              out_e, send_d, recv_d, wup_s, wup_r)

    nc.compile()
    return nc


def _body(nc, tc, xT_e, wq_e, wkv_e, wo_e, c2_e, s2_e, ppm_e, idm_e, tri_e,
          out_e, send_d, recv_d, wup_s, wup_r):
    from contextlib import ExitStack

    ctx = ExitStack()
    with ctx:
        consts = ctx.enter_context(tc.tile_pool(name="consts", bufs=1))
        work = ctx.enter_context(tc.tile_pool(name="work", bufs=1))
        ptp = ctx.enter_context(tc.tile_pool(name="pt", bufs=6))
        # PSUM budget is 8 banks of [128 x 2KB]; each pool holds bufs copies
        # of each distinct tag, so tags are shared aggressively:
        #   projp: tag "proj" (warmup + q + kv) x2     = 2 banks
        #   ropep: tag "rope" (qrot + krot)    x2      = 2 banks
        #   stp:   tag "st"   (scores + vt)    x2      = 2 banks
        #   otp:   tag "ot"   (PV accum, 2 heads) x2   = 2 banks
        projp_cm = tc.tile_pool(name="projp", bufs=2, space="PSUM")
        projp = projp_cm.__enter__()
        stp_cm = tc.tile_pool(name="stp", bufs=2, space="PSUM")
        stp = stp_cm.__enter__()
        otp_cm = tc.tile_pool(name="otp", bufs=2, space="PSUM")
        otp = otp_cm.__enter__()
        ropep_cm = tc.tile_pool(name="ropep", bufs=2, space="PSUM")
        ropep = ropep_cm.__enter__()

        # ---- warmup collective, first in collective program order ----
        wup_sb = consts.tile([NC_CORES, 1, 8], BF16, tag="wup")
        nc.vector.memset(wup_sb[:], 0.0)
        nc.sync.dma_start(out=wup_s.ap(), in_=wup_sb[:])
        nc.gpsimd.collective_compute(
            "AllToAll",
            mybir.AluOpType.bypass,
            replica_groups=[list(range(NC_CORES))],
            ins=[wup_s.ap().opt()],
            outs=[wup_r.ap().opt()],
        )

        # ---- const + weight loads (small, land first) ----
        ppm_sb = consts.tile([128, 128], BF16, tag="ppm")
        nc.sync.dma_start(out=ppm_sb[:], in_=ppm_e[:, :])
        idm_sb = consts.tile([128, 128], BF16, tag="idm")
        nc.sync.dma_start(out=idm_sb[:], in_=idm_e[:, :])
        tri_sb = consts.tile([128, 128], BF16, tag="tri")
        nc.sync.dma_start(out=tri_sb[:], in_=tri_e[:, :])
        wq_sb = consts.tile([128, NDC, HPC * HD], BF16, tag="wq")
        nc.sync.dma_start(out=wq_sb[:], in_=wq_e.ap())
        wkv_sb = consts.tile([128, NDC, 2 * HD], BF16, tag="wkv")
        nc.sync.dma_start(out=wkv_sb[:], in_=wkv_e.ap())
        c2_sb = consts.tile([128, S], BF16, tag="c2")
        nc.scalar.dma_start(out=c2_sb[:], in_=c2_e[:, :])
        s2_sb = consts.tile([128, S], BF16, tag="s2")
        nc.scalar.dma_start(out=s2_sb[:], in_=s2_e[:, :])

        # xT per 512-seq-chunk across 2 queues (scalar queue is busy later,
        # sync queue handles small consts then idles until the sends)
        xT_sb = consts.tile([128, NDC, S], BF16, tag="xT")
        for n in range(NCH):
            sl = slice(512 * n, 512 * (n + 1))
            eng = nc.sync if n % 2 == 0 else nc.gpsimd
            eng.dma_start(out=xT_sb[:, :, sl], in_=xT_e[:, :, sl])
        # wo load deferred to the gpsimd queue after xT (used only at the end)
        wo_sb = consts.tile([128, NDC, D], BF16, tag="wo")
        for i in range(NDC):
            nc.gpsimd.dma_start(out=wo_sb[:, i, :], in_=wo_e[:, i, :])

        # ---- PE warm-up: stream dummy matmuls while the xT DMA fills ----
        # (gets HAM past the cold window so projection runs at full clock)
        warm_ps = projp.tile([128, 512], F32, tag="proj")
        for r in range(10):
            nc.tensor.matmul(
                warm_ps[:, 0:128],
                lhsT=idm_sb[:],
                rhs=tri_sb[:],
                start=True,
                stop=True,
            )

        # ---- persistent SBUF tiles ----
        qrot_sb = work.tile([128, S], BF16, tag="qrot")  # h0 rows 0:64, h1 64:128
        krot2_sb = work.tile([128, S], BF16, tag="krot2")  # krot duplicated
        vT_sb = work.tile([64, S], BF16, tag="vT")
        vext_sb = work.tile([128, NSB, HD + 1], BF16, tag="vext")
        nc.vector.memset(vext_sb[:, :, HD : HD + 1], 1.0)
        stg_sb = work.tile([128, S], BF16, tag="stg")

        scale = 1.0 / np.sqrt(HD)

        def proj_chunk(n):
            sl = slice(512 * n, 512 * (n + 1))
            q_ps = projp.tile([128, 512], F32, tag="proj", name=f"q{n}")
            for i in range(NDC):
                nc.tensor.matmul(
                    q_ps[:],
                    lhsT=wq_sb[:, i, :],
                    rhs=xT_sb[:, i, sl],
                    start=(i == 0),
                    stop=(i == NDC - 1),
                )
            kv_ps = projp.tile([128, 512], F32, tag="proj", name=f"kv{n}")
            for i in range(NDC):
                nc.tensor.matmul(
                    kv_ps[:],
                    lhsT=wkv_sb[:, i, :],
                    rhs=xT_sb[:, i, sl],
                    start=(i == 0),
                    stop=(i == NDC - 1),
                )
            return q_ps, kv_ps

        rope_tmp = tc.tile_pool(name="ropet", bufs=2)
        ropet = rope_tmp.__enter__()

        def rope_chunk(n, q_ps, kv_ps):
            sl = slice(512 * n, 512 * (n + 1))
            # q rope: qc/qs on DVE, then 2 full-array matmuls accumulate
            qc = ropet.tile([128, 512], BF16, tag="qc", name=f"qc{n}")
            nc.vector.tensor_tensor(
                out=qc[:], in0=q_ps[:], in1=c2_sb[:, sl], op=mybir.AluOpType.mult
            )
            qs = ropet.tile([128, 512], BF16, tag="qs", name=f"qs{n}")
            nc.vector.tensor_tensor(
                out=qs[:], in0=q_ps[:], in1=s2_sb[:, sl], op=mybir.AluOpType.mult
            )
            qrot_ps = ropep.tile([128, 512], F32, tag="rope", name=f"qr{n}")
            nc.tensor.matmul(
                qrot_ps[:], lhsT=ppm_sb[:], rhs=qs[:], start=True, stop=False
            )
            nc.tensor.matmul(
                qrot_ps[:], lhsT=idm_sb[:], rhs=qc[:], start=False, stop=True
            )
            nc.scalar.copy(out=qrot_sb[:, sl], in_=qrot_ps[:])

            # k rope (64 rows) + v extraction
            kc = ropet.tile([64, 512], BF16, tag="kc", name=f"kc{n}")
            nc.vector.tensor_tensor(
                out=kc[:], in0=kv_ps[0:64, :], in1=c2_sb[0:64, sl],
                op=mybir.AluOpType.mult,
            )
            ks = ropet.tile([64, 512], BF16, tag="ks", name=f"ks{n}")
            nc.vector.tensor_tensor(
                out=ks[:], in0=kv_ps[0:64, :], in1=s2_sb[0:64, sl],
                op=mybir.AluOpType.mult,
            )
            nc.vector.tensor_copy(out=vT_sb[:, sl], in_=kv_ps[64:128, :])
            krot_ps = ropep.tile([64, 512], F32, tag="rope", name=f"kr{n}")
            nc.tensor.matmul(
                krot_ps[:], lhsT=ppm_sb[0:64, 0:64], rhs=ks[:], start=True, stop=False
            )
            nc.tensor.matmul(
                krot_ps[:], lhsT=idm_sb[0:64, 0:64], rhs=kc[:], start=False, stop=True
            )
            # duplicate krot to partitions 64..127 so head-1 score matmuls
            # land on PE row-groups 2-3 (concurrent with head 0)
            nc.scalar.copy(out=krot2_sb[0:64, sl], in_=krot_ps[:])
            nc.scalar.copy(out=krot2_sb[64:128, sl], in_=krot_ps[:])

            # V transpose for this chunk's 4 k-blocks
            vt_ps = stp.tile([128, 4, HD], BF16, tag="st", name=f"vt{n}")
            for j in range(4):
                b = 4 * n + j
                nc.tensor.transpose(
                    vt_ps[:, j, :],
                    vT_sb[:, 128 * b : 128 * (b + 1)],
                    idm_sb[0:64, 0:64],
                )
            nc.vector.tensor_copy(
                out=vext_sb[:, 4 * n : 4 * (n + 1), 0:HD], in_=vt_ps[:]
            )

        def attention_chunk(k):
            # q columns [512k, 512k+512); k-blocks 0..4k+3
            nblk = 4 * k + 4
            ot = [
                otp.tile([HD + 1, 512], F32, tag="ot", name=f"ot{h}_{k}")
                for h in range(HPC)
            ]
            pend = []  # blocks whose PV is not yet emitted

            def emit_pv(b, pt, qoff, w):
                for h in range(HPC):
                    nc.tensor.matmul(
                        ot[h][:, qoff : qoff + w],
                        lhsT=vext_sb[:, b, :],
                        rhs=pt[h][:, 0:w],
                        start=(b == 0),
                        stop=(b == nblk - 1),
                    )

            for b in range(nblk):
                qoff = max(0, 128 * b - 512 * k)
                w = 512 - qoff
                q0 = 512 * k + qoff
                st0 = stp.tile([128, 512], F32, tag="st", name=f"st0_{k}_{b}")
                st1 = stp.tile([128, 512], F32, tag="st", name=f"st1_{k}_{b}")
                kb = slice(128 * b, 128 * (b + 1))
                # two heads on disjoint PE quadrants (row-tiled, concurrent)
                nc.tensor.matmul(
                    st0[:, 0:w], lhsT=krot2_sb[0:64, kb],
                    rhs=qrot_sb[0:64, q0 : q0 + w], start=True, stop=True,
                )
                nc.tensor.matmul(
                    st1[:, 0:w], lhsT=krot2_sb[64:128, kb],
                    rhs=qrot_sb[64:128, q0 : q0 + w], start=True, stop=True,
                )
                # emit previous block's PV now: PE streams it while ACT exps b
                if pend:
                    emit_pv(*pend.pop())
                pt = [
                    ptp.tile([128, 512], BF16, tag="pt", name=f"pt{h}_{k}_{b}")
                    for h in range(HPC)
                ]
                for h, st in ((0, st0), (1, st1)):
                    nc.scalar.activation(
                        out=pt[h][:, 0:w], in_=st[:, 0:w],
                        func=mybir.ActivationFunctionType.Exp, scale=scale,
                    )
                if 128 * b >= 512 * k:
                    # diagonal block: mask its first 128 cols (kpos > q -> 0)
                    for h in range(HPC):
                        nc.vector.tensor_tensor(
                            out=pt[h][:, 0:128], in0=pt[h][:, 0:128],
                            in1=tri_sb[:], op=mybir.AluOpType.mult,
                        )
                pend.append((b, pt, qoff, w))
            while pend:
                emit_pv(*pend.pop())

            # normalize: stage[64h:64h+64, chunk] = ot[0:64] / ot[64]
            for h in range(HPC):
                den = work.tile([1, 512], F32, tag="den")
                nc.vector.tensor_copy(out=den[:], in_=ot[h][HD : HD + 1, :])
                rec = work.tile([1, 512], F32, tag="rec")
                nc.vector.reciprocal_approx_fast(out=rec[:], in_=den[:])
                bcr = work.tile([HD, 512], F32, tag="bcr")
                nc.gpsimd.partition_broadcast(bcr[:], rec[:])
                nc.vector.tensor_tensor(
                    out=stg_sb[64 * h : 64 * (h + 1), 512 * k : 512 * (k + 1)],
                    in0=ot[h][0:HD, :],
                    in1=bcr[:],
                    op=mybir.AluOpType.mult,
                )
            # ship this chunk's two destination slices to the send buffer
            for half in range(2):
                dst = 2 * k + half
                nc.sync.dma_start(
                    out=send_d.ap()[dst],
                    in_=stg_sb[:, 512 * k + SC * half : 512 * k + SC * (half + 1)],
                )

        # ---- software-pipelined emission ----
        # PE order: proj c0 | proj c1 | rope c0 | att c0 | proj c2 | rope c1 |
        #           att c1 | proj c3 | rope c2 | att c2 | rope c3 | att c3
        pq = {}
        pq[0] = proj_chunk(0)
        pq[1] = proj_chunk(1)
        rope_chunk(0, *pq.pop(0))
        attention_chunk(0)
        pq[2] = proj_chunk(2)
        rope_chunk(1, *pq.pop(1))
        attention_chunk(1)
        pq[3] = proj_chunk(3)
        rope_chunk(2, *pq.pop(2))
        attention_chunk(2)
        rope_chunk(3, *pq.pop(3))
        attention_chunk(3)

        # ---- AllToAll: stage [128, 2048] -> per-core [128, 8, 256] ----
        nc.gpsimd.collective_compute(
            "AllToAll",
            mybir.AluOpType.bypass,
            replica_groups=[list(range(NC_CORES))],
            ins=[send_d.ap().opt()],
            outs=[recv_d.ap().opt()],
        )
        at_sb = work.tile([128, NC_CORES, SC], BF16, tag="at")
        nc.sync.dma_start(
            out=at_sb[:],
            in_=recv_d.ap().rearrange("j p w -> p j w"),
        )

        # ---- output projection: out[s, :] = attn[s, :] @ Wo ----
        rope_tmp.__exit__(None, None, None)
        ropep_cm.__exit__(None, None, None)
        otp_cm.__exit__(None, None, None)
        stp_cm.__exit__(None, None, None)
        projp_cm.__exit__(None, None, None)
        pso = ctx.enter_context(tc.tile_pool(name="pso", bufs=1, space="PSUM"))
        op_ps = pso.tile([128, S], F32, tag="op")
        for m in range(SC // 128):
            for j in range(NC_CORES):
                for dn in range(2):
                    nc.tensor.matmul(
                        op_ps[:, 1024 * m + 512 * dn : 1024 * m + 512 * (dn + 1)],
                        lhsT=at_sb[:, j, 128 * m : 128 * (m + 1)],
                        rhs=wo_sb[:, j, 512 * dn : 512 * (dn + 1)],
                        start=(j == 0),
                        stop=(j == NC_CORES - 1),
                    )
        out_sb = work.tile([128, S], F32, tag="osb")
        nc.scalar.copy(out=out_sb[:], in_=op_ps[:])
        nc.sync.dma_start(
            out=out_e.ap().rearrange("(m p) d -> p m d", p=128),
            in_=out_sb[:].rearrange("p (m d) -> p m d", m=SC // 128),
        )


# ---------------- host side ----------------

_CACHE = {}


def _prep_consts():
    # ppm: lhsT of the signed half-swap M (per 64 block: [[0,-I],[I,0]])
    M = np.zeros((128, 128), np.float32)
    for hb in range(2):
        o = 64 * hb
        for j in range(HALF):
            M[o + j, o + HALF + j] = -1.0
            M[o + HALF + j, o + j] = 1.0
    ppm = M.T.astype(np_bf16)
    idm = np.eye(128, dtype=np_bf16)
    # tri[p, j] = 1 if j >= p (valid: sq >= sk within diagonal block)
    tri = (np.arange(128)[None, :] >= np.arange(128)[:, None]).astype(np_bf16)
    return ppm, idm, tri


def kernel(x, rope_cos, rope_sin, Wq, Wk, Wv, Wo):
    if "nc" not in _CACHE:
        _CACHE["nc"] = build_graph()
    nc = _CACHE["nc"]

    x2 = np.asarray(x, np.float32).reshape(S, D)
    xT = np.ascontiguousarray(x2.T).astype(np_bf16)
    cosT = np.asarray(rope_cos, np.float32).T  # [32, S]
    sinT = np.asarray(rope_sin, np.float32).T
    c2 = np.tile(cosT, (4, 1)).astype(np_bf16)  # [128, S]
    s2 = np.tile(sinT, (4, 1)).astype(np_bf16)
    ppm, idm, tri = _prep_consts()

    Wq = np.asarray(Wq, np.float32)
    Wk = np.asarray(Wk, np.float32)
    Wv = np.asarray(Wv, np.float32)
    Wo = np.asarray(Wo, np.float32)

    def chunked(w):  # [1024, X] -> [128, 8, X] (partition-major d-chunks)
        return np.ascontiguousarray(
            w.reshape(NDC, 128, -1).transpose(1, 0, 2)
        ).astype(np_bf16)

    # xT dram layout [128, NDC, S]: partition p, d-chunk i -> x[:, 128*i + p]
    xT3 = np.ascontiguousarray(
        x2.T.reshape(NDC, 128, S).transpose(1, 0, 2)
    ).astype(np_bf16)

    wo_b = chunked(Wo)
    in_maps = []
    for c in range(NC_CORES):
        kv = c // 2
        wq_c = chunked(Wq[:, HPC * HD * c : HPC * HD * (c + 1)])
        wkv_c = chunked(
            np.concatenate(
                [Wk[:, HD * kv : HD * (kv + 1)], Wv[:, HD * kv : HD * (kv + 1)]],
                axis=1,
            )
        )
        in_maps.append(
            {
                "xT": xT3,
                "wq": wq_c,
                "wkv": wkv_c,
                "wo": wo_b,
                "c2": c2,
                "s2": s2,
                "ppm": ppm,
                "idm": idm,
                "tri": tri,
            }
        )

    res = run_bass_kernel_spmd(nc, in_maps, core_ids=list(range(NC_CORES)))
    out = np.concatenate([res.results[c]["out"] for c in range(NC_CORES)], axis=0)
    return out.reshape(B, S, D).astype(np.float32)
